# revision 9
# baseline (speedup 1.0000x reference)
"""CG coupler (segment_reduce) Trainium2 kernel.

out[b, ro[t]] += x1[b, r1[t]] * x2[b, r2[t]] * cg[t]   for t in range(T)

The CG index tables produced by the coupler have a rigid structure: T splits
into runs of exactly 128 consecutive indices (the channel dimension) that are
128-aligned in all three tensors, with a constant coefficient per run.  Each
run is therefore one dense slot-level FMA:

    out[:, so*128:(so+1)*128] += c * x1[:, s1*128:...] * x2[:, s2*128:...]

We detect that structure from the runtime index arrays on the host and bake it
into the Bass program.  Per core (batch is data-parallel across 8 cores):

  - inputs stream in per (pass, column-group) so products can start early
  - the distinct (s1,s2) slot products are computed in fp32, split between
    the DVE and Pool engines by a running load-balance
  - per-term scaled-identity matmuls accumulate into PSUM; operands are
    bitcast to float32r, which the PE runs at 1 cycle/row for moving size
    >= 256 (plain fp32 runs at 4 cycles/row)
  - matmuls for one output slot are issued contiguously (start on first,
    stop on last), so no PSUM-zeroing matmuls are needed
  - the Act engine evacuates each PSUM bank to SBUF; the bank's columns are
    then DMA'd straight to DRAM
"""

import sys

for _p in ("/opt/trn_rl_repo",):
    if _p not in sys.path:
        sys.path.insert(0, _p)

from contextlib import ExitStack

import numpy as np

import concourse.bass as bass
import concourse.mybir as mybir
import concourse.tile as tile
from concourse import bacc
from concourse.bass_utils import run_bass_kernel_spmd

N_CORES = 8
P = 128
F32 = mybir.dt.float32
F32R = mybir.dt.float32r

_CACHE: dict = {}


def _detect_plan(r1, r2, ro, cg, in_dim, out_dim):
    """Return list of (s1, s2, so, c) slot terms, or None if the index tables
    don't have the aligned 128-run structure."""
    T = len(cg)
    if T % P != 0 or len(r1) != T or len(r2) != T or len(ro) != T:
        return None
    d1 = np.diff(r1)
    d2 = np.diff(r2)
    do = np.diff(ro)
    brk = np.where(~((d1 == 1) & (d2 == 1) & (do == 1)))[0] + 1
    starts = np.concatenate([[0], brk])
    ends = np.concatenate([brk, [T]])
    if not np.all(ends - starts == P):
        return None
    a0, b0, o0 = r1[starts], r2[starts], ro[starts]
    if (a0 % P).any() or (b0 % P).any() or (o0 % P).any():
        return None
    if a0.max() + P > in_dim or b0.max() + P > in_dim or o0.max() + P > out_dim:
        return None
    cg2 = np.asarray(cg).reshape(-1, P)
    if not np.all(cg2 == cg2[:, :1]):
        return None
    return list(
        zip(
            (a0 // P).tolist(),
            (b0 // P).tolist(),
            (o0 // P).tolist(),
            cg2[:, 0].astype(np.float64).tolist(),
        )
    )


def _numpy_fallback(x1, x2, cg, r1, r2, ro, out_dim):
    out = np.zeros((x1.shape[0], out_dim), dtype=x1.dtype)
    prod = x1[:, r1] * x2[:, r2] * cg[None, :].astype(x1.dtype)
    np.add.at(out, (slice(None), ro), prod)
    return out


# cost-model engine-busy estimates (ns) for one [128, 256] tensor_tensor
_DVE_TT_NS = 316.0
_POOL_TT_NS = 530.0
_POOL_SETUP_NS = 290.0  # memset + affine_select per scaled identity

SLOTS_PER_GROUP = 4  # column-group granularity for input DMA (512 cols)


def _build_program(terms, b_shard, in_dim, out_dim):
    """Build the per-core Bass program. Every core runs the same program on
    its own batch shard (data-parallel, no collectives)."""
    nblk = b_shard // P
    assert nblk % 2 == 0
    n_passes = nblk // 2
    n_so = out_dim // P
    n_s_in = in_dim // P

    # load chunking: fine-grained leading chunks so the first pair products
    # (low slots) can start ~2us in, coarser after
    def pass_chunks(ps):
        if ps == 0:
            singles = min(4, n_s_in)
            chunks = [[s] for s in range(singles)]
            s = singles
        else:
            chunks, s = [], 0
        while s < n_s_in:
            e = min(s + SLOTS_PER_GROUP, n_s_in)
            chunks.append(list(range(s, e)))
            s = e
        return chunks

    # estimated DMA completion time per (pass, slot) assuming serial DMA
    # engines at ~0.36 B/ns starting ~1.4us in (HWDGE pipeline fill)
    load_done = {}
    t = 1400.0
    for ps in range(n_passes):
        for chunk in pass_chunks(ps):
            dur = 2 * P * len(chunk) * P * 4 / 0.36  # both row-blocks, ns
            t += dur  # x1 chunk
            t += dur  # x2 chunk
            for s in chunk:
                load_done[(ps, s)] = t

    # distinct (s1, s2) pairs; per-pass greedy engine assignment by
    # projected completion time
    pairs: dict = {}
    for s1, s2, so, c in terms:
        pairs.setdefault((s1, s2), []).append((so, c))

    slot_of = {}  # (pass, pair) -> terms ordering comes later
    cvals_first_use = {}

    nc = bacc.Bacc("TRN2", target_bir_lowering=False, debug=False)
    x1d = nc.dram_tensor("x1", [b_shard, in_dim], F32, kind="ExternalInput").ap()
    x2d = nc.dram_tensor("x2", [b_shard, in_dim], F32, kind="ExternalInput").ap()
    outd = nc.dram_tensor("out", [b_shard, out_dim], F32, kind="ExternalOutput").ap()

    with tile.TileContext(nc) as tc, ExitStack() as ctx:
        const_p = ctx.enter_context(tc.tile_pool(name="const", bufs=1))
        big_p = ctx.enter_context(tc.tile_pool(name="big", bufs=1))
        prod_p = ctx.enter_context(tc.tile_pool(name="prod", bufs=88))
        psum_p = ctx.enter_context(tc.tile_pool(name="psum", bufs=8, space="PSUM"))

        # fp32 unit identity (Pool). Scaled f32r copies are made on Act below,
        # ordered by first use (gpsimd can't legally write f32r; Act rounds).
        ident = const_p.tile([P, P], F32, tag="ident")
        nc.gpsimd.memset(ident[:], 0.0)
        nc.gpsimd.affine_select(
            out=ident[:],
            in_=ident[:],
            compare_op=mybir.AluOpType.not_equal,
            fill=1.0,
            base=0,
            pattern=[[-1, P]],
            channel_multiplier=1,
        )

        X1 = big_p.tile([P, nblk * in_dim], F32, tag="X1")
        X2 = big_p.tile([P, nblk * in_dim], F32, tag="X2")
        OUT = big_p.tile([P, nblk * out_dim], F32, tag="OUT")
        X1r = X1[:].rearrange("p (blk f) -> p blk f", blk=nblk)
        X2r = X2[:].rearrange("p (blk f) -> p blk f", blk=nblk)
        OUTr = OUT[:].rearrange("p (blk f) -> p blk f", blk=nblk)

        for ps in range(n_passes):
            rows = slice(ps * 2 * P, (ps + 1) * 2 * P)
            for chunk in pass_chunks(ps):
                cols = slice(chunk[0] * P, (chunk[-1] + 1) * P)
                nc.sync.dma_start(
                    out=X1r[:, 2 * ps : 2 * ps + 2, cols],
                    in_=x1d[rows, cols].rearrange("(blk p) f -> p blk f", p=P),
                )
                nc.sync.dma_start(
                    out=X2r[:, 2 * ps : 2 * ps + 2, cols],
                    in_=x2d[rows, cols].rearrange("(blk p) f -> p blk f", p=P),
                )

        # plan products and term order for every pass up front (host-side),
        # so scaled identities can be built in first-use order
        eng_vt = {"dve": 0.0, "pool": _POOL_SETUP_NS}
        plan = []  # per pass: (prod_assign list, term list)
        for ps in range(n_passes):
            ready = {
                p: max(load_done[(ps, p[0])], load_done[(ps, p[1])]) for p in pairs
            }
            order = sorted(pairs, key=lambda p: (ready[p], p))
            assign = []
            t_done = {}
            for p in order:
                fin_d = max(ready[p], eng_vt["dve"]) + _DVE_TT_NS
                fin_p = max(ready[p], eng_vt["pool"]) + _POOL_TT_NS
                if fin_d <= fin_p:
                    eng_vt["dve"] = fin_d
                    assign.append((p, "dve"))
                    t_done[p] = fin_d
                else:
                    eng_vt["pool"] = fin_p
                    assign.append((p, "pool"))
                    t_done[p] = fin_p
            term_list = []  # (t_done, so, pair, c)
            for (s1, s2), tl in pairs.items():
                for so, c in tl:
                    term_list.append((t_done[(s1, s2)], so, (s1, s2), c))
            term_list.sort()
            plan.append((assign, term_list))
            for _, so, p, c in term_list:
                cvals_first_use.setdefault(c, len(cvals_first_use))

        # scaled f32r identities on Act, in first-use order
        sids = {}
        for c, i in sorted(cvals_first_use.items(), key=lambda kv: kv[1]):
            t_ = const_p.tile([P, P], F32R, tag=f"sid{i}")
            nc.scalar.activation(
                out=t_[:],
                in_=ident[:],
                func=mybir.ActivationFunctionType.Copy,
                scale=float(c),
            )
            sids[c] = t_

        n_banks = (n_so + 1) // 2

        for ps in range(n_passes):
            assign, term_list = plan[ps]

            banks = []
            for k in range(n_banks):
                bk = psum_p.tile([P, 512], F32, tag="bank")
                banks.append(bk)

            prods = {}
            for p, eng_name in assign:
                pr = prod_p.tile([P, 2 * P], F32R, tag="prod")
                eng = nc.vector if eng_name == "dve" else nc.gpsimd
                eng.tensor_tensor(
                    out=pr[:].rearrange("p (b f) -> p b f", b=2),
                    in0=X1r[:, 2 * ps : 2 * ps + 2, p[0] * P : (p[0] + 1) * P],
                    in1=X2r[:, 2 * ps : 2 * ps + 2, p[1] * P : (p[1] + 1) * P],
                    op=mybir.AluOpType.mult,
                )
                prods[p] = pr

            # contiguous per-slot accumulation groups, slots ordered by the
            # estimated completion time of their last product
            slot_key = {}
            for td, so, p, c in term_list:
                slot_key[so] = max(slot_key.get(so, 0.0), td)
            slot_terms = {}
            for td, so, p, c in term_list:
                slot_terms.setdefault(so, []).append((td, p, c))
            n_in_bank_done = [0] * n_banks
            for so in sorted(slot_terms, key=lambda s: (slot_key[s], s)):
                k, so_l = divmod(so, 2)
                tl = sorted(slot_terms[so])
                for i, (_, p, c) in enumerate(tl):
                    nc.tensor.matmul(
                        out=banks[k][:, so_l * 256 : so_l * 256 + 256],
                        lhsT=sids[c][:],
                        rhs=prods[p][:],
                        start=(i == 0),
                        stop=(i == len(tl) - 1),
                    )
                n_in_bank_done[k] += 1
                if n_in_bank_done[k] == (2 if 2 * k + 1 < n_so else 1):
                    n_in_bank = 2 if 2 * k + 1 < n_so else 1
                    nc.scalar.copy(
                        out=OUTr[
                            :, 2 * ps : 2 * ps + 2, 2 * k * P : (2 * k + n_in_bank) * P
                        ].rearrange("p b (s f) -> p s b f", s=n_in_bank),
                        in_=banks[k][:, : n_in_bank * 256].rearrange(
                            "p (s b f) -> p s b f", s=n_in_bank, b=2
                        ),
                    )
                    nc.sync.dma_start(
                        out=outd[
                            ps * 2 * P : (ps + 1) * 2 * P,
                            2 * k * P : (2 * k + n_in_bank) * P,
                        ].rearrange("(blk p) f -> p blk f", p=P),
                        in_=OUTr[
                            :, 2 * ps : 2 * ps + 2, 2 * k * P : (2 * k + n_in_bank) * P
                        ],
                    )

    nc.finalize()  # run the bacc pass pipeline (wait splitting, regalloc, ...)
    return nc


def kernel(x1, x2, cg_tilde, repids_in1, repids_in2, repids_out, out_dim):
    x1 = np.ascontiguousarray(np.asarray(x1, dtype=np.float32))
    x2 = np.ascontiguousarray(np.asarray(x2, dtype=np.float32))
    cg = np.asarray(cg_tilde, dtype=np.float32)
    r1 = np.asarray(repids_in1).astype(np.int64)
    r2 = np.asarray(repids_in2).astype(np.int64)
    ro = np.asarray(repids_out).astype(np.int64)
    out_dim = int(np.asarray(out_dim))

    B, in_dim = x1.shape
    terms = None
    if (
        B % (N_CORES * 2 * P) == 0
        and in_dim % P == 0
        and out_dim % P == 0
        and x2.shape == x1.shape
    ):
        terms = _detect_plan(r1, r2, ro, cg, in_dim, out_dim)
    if terms is None:
        return _numpy_fallback(x1, x2, cg, r1, r2, ro, out_dim)

    b_shard = B // N_CORES
    key = (B, in_dim, out_dim, np.asarray(terms, dtype=np.float64).tobytes())
    nc = _CACHE.get(key)
    if nc is None:
        nc = _build_program(terms, b_shard, in_dim, out_dim)
        _CACHE[key] = nc

    in_maps = [
        {
            "x1": x1[i * b_shard : (i + 1) * b_shard],
            "x2": x2[i * b_shard : (i + 1) * b_shard],
        }
        for i in range(N_CORES)
    ]
    res = run_bass_kernel_spmd(nc, in_maps, core_ids=list(range(N_CORES)))
    return np.concatenate([res.results[i]["out"] for i in range(N_CORES)], axis=0)


# revision 19
# speedup vs baseline: 1.0700x; 1.0700x over previous
"""CG coupler (segment_reduce) Trainium2 kernel.

out[b, ro[t]] += x1[b, r1[t]] * x2[b, r2[t]] * cg[t]   for t in range(T)

The CG index tables produced by the coupler have a rigid structure: T splits
into runs of exactly 128 consecutive indices (the channel dimension) that are
128-aligned in all three tensors, with a constant coefficient per run.  Each
run is therefore one dense slot-level FMA:

    out[:, so*128:(so+1)*128] += c * x1[:, s1*128:...] * x2[:, s2*128:...]

We detect that structure from the runtime index arrays on the host and bake it
into the Bass program.  Per core (batch is data-parallel across 8 cores):

  - inputs stream in per (pass, column-group) so products can start early
  - the distinct (s1,s2) slot products are computed in fp32, split between
    the DVE and Pool engines by a running load-balance
  - per-term scaled-identity matmuls accumulate into PSUM; operands are
    bitcast to float32r, which the PE runs at 1 cycle/row for moving size
    >= 256 (plain fp32 runs at 4 cycles/row)
  - matmuls for one output slot are issued contiguously (start on first,
    stop on last), so no PSUM-zeroing matmuls are needed
  - the Act engine evacuates each PSUM bank to SBUF; the bank's columns are
    then DMA'd straight to DRAM
"""

import sys

for _p in ("/opt/trn_rl_repo",):
    if _p not in sys.path:
        sys.path.insert(0, _p)

from contextlib import ExitStack

import numpy as np

import concourse.bass as bass
import concourse.mybir as mybir
import concourse.tile as tile
from concourse import bacc
from concourse.bass_utils import run_bass_kernel_spmd

N_CORES = 8
P = 128
F32 = mybir.dt.float32
F32R = mybir.dt.float32r
BF16 = mybir.dt.bfloat16

_CACHE: dict = {}


def _detect_plan(r1, r2, ro, cg, in_dim, out_dim):
    """Return list of (s1, s2, so, c) slot terms, or None if the index tables
    don't have the aligned 128-run structure."""
    T = len(cg)
    if T % P != 0 or len(r1) != T or len(r2) != T or len(ro) != T:
        return None
    d1 = np.diff(r1)
    d2 = np.diff(r2)
    do = np.diff(ro)
    brk = np.where(~((d1 == 1) & (d2 == 1) & (do == 1)))[0] + 1
    starts = np.concatenate([[0], brk])
    ends = np.concatenate([brk, [T]])
    if not np.all(ends - starts == P):
        return None
    a0, b0, o0 = r1[starts], r2[starts], ro[starts]
    if (a0 % P).any() or (b0 % P).any() or (o0 % P).any():
        return None
    if a0.max() + P > in_dim or b0.max() + P > in_dim or o0.max() + P > out_dim:
        return None
    cg2 = np.asarray(cg).reshape(-1, P)
    if not np.all(cg2 == cg2[:, :1]):
        return None
    return list(
        zip(
            (a0 // P).tolist(),
            (b0 // P).tolist(),
            (o0 // P).tolist(),
            cg2[:, 0].astype(np.float64).tolist(),
        )
    )


def _numpy_fallback(x1, x2, cg, r1, r2, ro, out_dim):
    out = np.zeros((x1.shape[0], out_dim), dtype=x1.dtype)
    prod = x1[:, r1] * x2[:, r2] * cg[None, :].astype(x1.dtype)
    np.add.at(out, (slice(None), ro), prod)
    return out


# cost-model engine-busy estimates (ns) for [128, N]-free elementwise ops
def _dve_tt(free):  # bf16 tensor_tensor, 2x_1p
    return free * 1.0417 * 0.5 + 60.0


def _pool_tt(free):  # tensor_tensor; Pool gets no DVE 2x modes, 0.42 sw eff
    return free * 0.8333 / 0.42 + 30.0


def _dve_conv(free):  # fp32->bf16 tensor_copy, 2x_2p
    return free * 1.0417 * 0.5 + 60.0


def _act_conv(free):  # fp32->bf16 activation copy
    return free * 0.8333 + 185.0


def _pool_conv(free):  # fp32->bf16 copy on gpsimd (0.6 default sw efficiency)
    return free * 0.8333 / 0.6 + 30.0


_PLAN_CFG = {"act_vt0": 2200.0, "dve_conv_shadow": 1.0, "pool_conv": True}


_ACT_SID_NS = 292.0
_ACT_EVAC_NS = 612.0
_MM_NS = 107.0  # bf16 matmul, 256 moving rows

SLOTS_PER_GROUP = 4  # column-group granularity for input DMA (512 cols)


def _mirror_plan(pairs):
    """Split terms into direct terms and mirror-combined terms.

    Returns (direct, combined, combines) where
      direct:   list of (pair, so, c)             -> rhs = product(pair)
      combined: list of (upair, sign, so, c)      -> rhs = S_sign(upair)
      combines: list of (upair, sign)             -> S_sign = pr_ab + sign*pr_ba
    """
    direct, combined, combines = [], [], set()
    done = set()
    for (a, b), tl in pairs.items():
        if (a, b) in done:
            continue
        if a == b or (b, a) not in pairs:
            done.add((a, b))
            for so, c in tl:
                direct.append(((a, b), so, c))
            continue
        d1 = dict(tl)
        d2 = dict(pairs[(b, a)])
        done.add((a, b))
        done.add((b, a))
        if set(d1) != set(d2):
            for so, c in d1.items():
                direct.append(((a, b), so, c))
            for so, c in d2.items():
                direct.append(((b, a), so, c))
            continue
        ok = all(abs(abs(d1[so]) - abs(d2[so])) <= 1e-5 * abs(d1[so]) for so in d1)
        if not ok:
            for so, c in d1.items():
                direct.append(((a, b), so, c))
            for so, c in d2.items():
                direct.append(((b, a), so, c))
            continue
        up = (a, b) if a < b else (b, a)
        da, db = (d1, d2) if a < b else (d2, d1)
        for so in da:
            sign = 1 if da[so] * db[so] > 0 else -1
            combined.append((up, sign, so, da[so]))
            combines.add((up, sign))
    return direct, combined, sorted(combines)


def _build_program(terms, b_shard, in_dim, out_dim):
    """Build the per-core Bass program. Every core runs the same program on
    its own batch shard (data-parallel, no collectives).

    v7: inputs are converted to bf16 per chunk (staging pool), pair products
    and mirror-combines run in bf16 on DVE+Pool (2x modes), per-term
    scaled-identity bf16 matmuls accumulate in PSUM (1 cycle/row), and
    mirrored pairs are folded (c*pr_ab + (+-c)*pr_ba = c*(pr_ab +- pr_ba))
    to halve the matmul count.  All engine queues are emitted in
    estimated-execution-time order from a host-side list-scheduling plan.
    """
    nblk = b_shard // P
    assert nblk % 2 == 0
    n_passes = nblk // 2
    n_so = out_dim // P
    n_s_in = in_dim // P
    n_banks = (n_so + 1) // 2

    def pass_chunks(ps):
        if ps == 0:
            singles = min(4, n_s_in)
            chunks = [[s] for s in range(singles)]
            s = singles
        else:
            chunks, s = [], 0
        while s < n_s_in:
            e = min(s + SLOTS_PER_GROUP, n_s_in)
            chunks.append(list(range(s, e)))
            s = e
        return chunks

    # --- host-side plan -----------------------------------------------------
    # load completion estimates (serial DMA engines, ~0.36 B/ns, ~1.4us fill)
    load_done = {}  # (ps, tensor, chunk_idx) -> t ; also (ps, slot) -> t
    t = 1400.0
    for ps in range(n_passes):
        for ci, chunk in enumerate(pass_chunks(ps)):
            dur = 2 * P * len(chunk) * P * 4 / 0.36
            t += dur
            load_done[(ps, 0, ci)] = t
            t += dur
            load_done[(ps, 1, ci)] = t

    pairs: dict = {}
    for s1, s2, so, c in terms:
        pairs.setdefault((s1, s2), []).append((so, c))
    # mirror-combining is net-negative here: each combine costs a DVE/Pool
    # tensor_tensor (~193/508 ns) to save one 107 ns PE matmul, and PE is not
    # the binding engine. Keep all terms direct.
    direct = [(p, so, c) for p, tl in pairs.items() for so, c in tl]
    combined, combines = [], []

    # unified dependency-driven list scheduler: convs and products are
    # dispatched in global ready order (interleaved!), each to the engine
    # that finishes it earliest. Scheduling convs phase-first would push one
    # engine's clock far ahead and starve it of product work.
    import heapq as _hq

    vt = {"dve": 0.0, "pool": 300.0, "act": _PLAN_CFG["act_vt0"]}
    done = {}
    assign = {}
    conv_done = {}  # (ps, tensor, slot) -> t
    heap = []
    for ps in range(n_passes):
        for ci, chunk in enumerate(pass_chunks(ps)):
            free = 2 * len(chunk) * P
            for tn in (0, 1):
                _hq.heappush(
                    heap,
                    (load_done[(ps, tn, ci)], 0, ("conv", ps, tn, ci),
                     {"free": free, "chunk": chunk}),
                )
    prod_deps = {}
    for ps in range(n_passes):
        for p in pairs:
            prod_deps[("prod", ps, p)] = 2
    chunk_idx = {}
    for ps in range(n_passes):
        for ci, chunk in enumerate(pass_chunks(ps)):
            for s in chunk:
                chunk_idx[(ps, s)] = ci
    waiters = {}
    for ps in range(n_passes):
        for p in pairs:
            waiters.setdefault(("conv", ps, 0, chunk_idx[(ps, p[0])]), []).append(
                ("prod", ps, p)
            )
            waiters.setdefault(("conv", ps, 1, chunk_idx[(ps, p[1])]), []).append(
                ("prod", ps, p)
            )
    prod_ready = {k: 0.0 for k in prod_deps}
    seq = 1
    while heap:
        ready, _, key, meta = _hq.heappop(heap)
        if key[0] == "conv":
            cand = [
                ("act", max(ready, vt["act"]) + _act_conv(meta["free"]),
                 _act_conv(meta["free"])),
                ("dve",
                 max(ready, vt["dve"])
                 + _dve_conv(meta["free"]) * _PLAN_CFG["dve_conv_shadow"],
                 _dve_conv(meta["free"])),
            ]
            if _PLAN_CFG["pool_conv"]:
                cand.append(
                    ("pool", max(ready, vt["pool"]) + _pool_conv(meta["free"]),
                     _pool_conv(meta["free"]))
                )
        else:
            cand = (
                ("dve", max(ready, vt["dve"]) + _dve_tt(2 * P), _dve_tt(2 * P)),
                ("pool", max(ready, vt["pool"]) + _pool_tt(2 * P), _pool_tt(2 * P)),
            )
        eng, fin, cost = min(cand, key=lambda c: c[1])
        fin = max(ready, vt[eng]) + cost
        vt[eng] = fin
        assign[key] = eng
        done[key] = fin
        if key[0] == "conv":
            _, ps, tn, ci = key
            for s in pass_chunks(ps)[ci]:
                conv_done[(ps, tn, s)] = fin
            for w in waiters.get(key, []):
                prod_ready[w] = max(prod_ready[w], fin)
                prod_deps[w] -= 1
                if prod_deps[w] == 0:
                    seq += 1
                    _hq.heappush(heap, (prod_ready[w], seq, w, None))

    # per-pass slot groups: rhs item for each term, slot ordered by the
    # latest rhs completion; PE progress estimate gives evac/store order
    slot_plans = []  # per pass: list of (slot, [(rhs_key, c), ...])
    cvals_first_use = {}
    evac_est = []  # (est, ps, bank)
    for ps in range(n_passes):
        rhs_of = {}
        for p, so, c in direct:
            rhs_of.setdefault(so, []).append((("prod", ps, p), c))
        for up, sign, so, c in combined:
            rhs_of.setdefault(so, []).append((("comb", ps, up, sign), c))
        key_of = {
            so: max(done[rk] for rk, _ in tl) for so, tl in rhs_of.items()
        }
        order = sorted(rhs_of, key=lambda so: (key_of[so], so))
        slot_plan = []
        pe_vt = 0.0
        bank_seen = [0] * n_banks
        for so in order:
            tl = sorted(rhs_of[so], key=lambda rc: done[rc[0]])
            slot_plan.append((so, tl))
            for rk, c in tl:
                pe_vt = max(pe_vt, done[rk]) + _MM_NS
                cvals_first_use.setdefault(c, len(cvals_first_use))
            k = so // 2
            bank_seen[k] += 1
            if bank_seen[k] == (2 if 2 * k + 1 < n_so else 1):
                evac_est.append((pe_vt + 100.0, ps, k))
        slot_plans.append(slot_plan)

    # --- emit -------------------------------------------------------------
    # The Tile framework derives dependencies from program order, so the
    # emission stream must be causally ordered (producers before consumers).
    # Emit a single global stream: a heap ordered by estimated start time,
    # popping events only once their dependencies have been emitted.
    import heapq

    nc = bacc.Bacc("TRN2", target_bir_lowering=False, debug=False)
    x1d = nc.dram_tensor("x1", [b_shard, in_dim], F32, kind="ExternalInput").ap()
    x2d = nc.dram_tensor("x2", [b_shard, in_dim], F32, kind="ExternalInput").ap()
    outd = nc.dram_tensor("out", [b_shard, out_dim], F32, kind="ExternalOutput").ap()

    with tile.TileContext(nc) as tc, ExitStack() as ctx:
        const_p = ctx.enter_context(tc.tile_pool(name="const", bufs=1))
        big_p = ctx.enter_context(tc.tile_pool(name="big", bufs=1))
        stage_p = ctx.enter_context(tc.tile_pool(name="stage", bufs=12))
        prod_p = ctx.enter_context(tc.tile_pool(name="prod", bufs=96))
        psum_p = ctx.enter_context(tc.tile_pool(name="psum", bufs=8, space="PSUM"))

        ident = const_p.tile([P, P], F32, tag="ident")
        nc.gpsimd.memset(ident[:], 0.0)
        nc.gpsimd.affine_select(
            out=ident[:],
            in_=ident[:],
            compare_op=mybir.AluOpType.not_equal,
            fill=1.0,
            base=0,
            pattern=[[-1, P]],
            channel_multiplier=1,
        )

        X1B = big_p.tile([P, nblk * in_dim], BF16, tag="X1B")
        X2B = big_p.tile([P, nblk * in_dim], BF16, tag="X2B")
        OUT = big_p.tile([P, nblk * out_dim], F32, tag="OUT")
        XBr = [
            X1B[:].rearrange("p (blk f) -> p blk f", blk=nblk),
            X2B[:].rearrange("p (blk f) -> p blk f", blk=nblk),
        ]
        OUTr = OUT[:].rearrange("p (blk f) -> p blk f", blk=nblk)

        # PSUM bank tiles, pass-major so pass p+1's bank k aliases pass p's
        banks = {}
        for ps in range(n_passes):
            for k in range(n_banks):
                bk = psum_p.tile([P, 512], F32, tag="bank")
                banks[(ps, k)] = bk

        sids = {}
        for c, i in sorted(cvals_first_use.items(), key=lambda kv: kv[1]):
            t_ = const_p.tile([P, P], BF16, tag=f"sid{i}")
            sids[c] = t_

        # ---- event graph ---------------------------------------------------
        raw_events = []  # (eid, est, deps, emit); deps wired after collection

        def add(eid, est, deps, emit):
            raw_events.append((eid, est, deps, emit))

        chunk_of_slot = {}
        for ps in range(n_passes):
            for ci, chunk in enumerate(pass_chunks(ps)):
                for s in chunk:
                    chunk_of_slot[(ps, s)] = ci

        # sids: emit early, ordered by first use (Act)
        for c, i in sorted(cvals_first_use.items(), key=lambda kv: kv[1]):
            def em_sid(c=c):
                nc.scalar.activation(
                    out=sids[c][:],
                    in_=ident[:],
                    func=mybir.ActivationFunctionType.Copy,
                    scale=float(c),
                )
            add(("sid", c), 500.0 + 40.0 * i, [], em_sid)

        # loads (SP queue); explicit WAR dep on the conv 12 loads back
        load_seq = []
        for ps in range(n_passes):
            for ci, chunk in enumerate(pass_chunks(ps)):
                for tn in (0, 1):
                    load_seq.append((ps, ci, tn))
        stages = {}
        for gi, (ps, ci, tn) in enumerate(load_seq):
            chunk = pass_chunks(ps)[ci]
            cols = slice(chunk[0] * P, (chunk[-1] + 1) * P)
            w = (chunk[-1] + 1 - chunk[0]) * P
            rows = slice(ps * 2 * P, (ps + 1) * 2 * P)
            xd = x1d if tn == 0 else x2d
            dur = 2 * P * w * 4 / 0.36
            deps = []
            if gi >= 12:
                deps.append(("conv",) + load_seq[gi - 12])
            def em_load(ps=ps, ci=ci, tn=tn, cols=cols, w=w, rows=rows, xd=xd):
                st = stage_p.tile([P, 2, SLOTS_PER_GROUP * P], F32, tag="stage")
                nc.sync.dma_start(
                    out=st[:, :, :w],
                    in_=xd[rows, cols].rearrange("(blk p) f -> p blk f", p=P),
                )
                stages[(ps, tn, ci)] = st
            add(("load", ps, ci, tn), load_done[(ps, tn, ci)] - dur, deps, em_load)

        # conversions fp32 -> bf16 into the big bf16 tiles
        for ps in range(n_passes):
            for ci, chunk in enumerate(pass_chunks(ps)):
                cols = slice(chunk[0] * P, (chunk[-1] + 1) * P)
                w = (chunk[-1] + 1 - chunk[0]) * P
                for tn in (0, 1):
                    key = ("conv", ps, ci, tn)
                    eng = assign[("conv", ps, tn, ci)]
                    def em_conv(ps=ps, ci=ci, tn=tn, cols=cols, w=w, eng=eng):
                        st = stages[(ps, tn, ci)]
                        out_ap = XBr[tn][:, 2 * ps : 2 * ps + 2, cols]
                        if eng == "act":
                            nc.scalar.copy(out=out_ap, in_=st[:, :, :w])
                        elif eng == "pool":
                            nc.gpsimd.tensor_copy(out=out_ap, in_=st[:, :, :w])
                        else:
                            nc.vector.tensor_copy(out=out_ap, in_=st[:, :, :w])
                    add(key, done[("conv", ps, tn, ci)],
                        [("load", ps, ci, tn)], em_conv)

        # pair products (DVE / Pool per plan)
        tiles = {}
        for ps in range(n_passes):
            for p in pairs:
                key = ("prod", ps, p)
                deps = [
                    ("conv", ps, chunk_of_slot[(ps, p[0])], 0),
                    ("conv", ps, chunk_of_slot[(ps, p[1])], 1),
                ]
                eng_name = assign[key]
                def em_prod(ps=ps, p=p, eng_name=eng_name, key=key):
                    pr = prod_p.tile([P, 2 * P], BF16, tag="prod")
                    eng = nc.vector if eng_name == "dve" else nc.gpsimd
                    eng.tensor_tensor(
                        out=pr[:].rearrange("p (b f) -> p b f", b=2),
                        in0=XBr[0][:, 2 * ps : 2 * ps + 2, p[0] * P : (p[0] + 1) * P],
                        in1=XBr[1][:, 2 * ps : 2 * ps + 2, p[1] * P : (p[1] + 1) * P],
                        op=mybir.AluOpType.mult,
                    )
                    tiles[key] = pr
                add(key, done[key] - _dve_tt(2 * P), deps, em_prod)

        # per-slot matmul groups, evacs, stores
        for ps in range(n_passes):
            for so, tl in slot_plans[ps]:
                k, so_l = divmod(so, 2)
                deps = [rk for rk, _ in tl]
                deps += [("sid", c) for _, c in tl]
                if ps > 0:
                    deps.append(("evac", ps - 1, k))
                def em_slot(ps=ps, so=so, tl=tl, k=k, so_l=so_l):
                    for i, (rk, c) in enumerate(tl):
                        nc.tensor.matmul(
                            out=banks[(ps, k)][:, so_l * 256 : so_l * 256 + 256],
                            lhsT=sids[c][:],
                            rhs=tiles[rk][:],
                            start=(i == 0),
                            stop=(i == len(tl) - 1),
                        )
                add(("slot", ps, so), max(done[rk] for rk, _ in tl),
                    deps, em_slot)
        for est, ps, k in evac_est:
            n_in_bank = 2 if 2 * k + 1 < n_so else 1
            deps = [("slot", ps, 2 * k)]
            if n_in_bank == 2:
                deps.append(("slot", ps, 2 * k + 1))
            def em_evac(ps=ps, k=k, n_in_bank=n_in_bank):
                nc.scalar.copy(
                    out=OUTr[
                        :, 2 * ps : 2 * ps + 2, 2 * k * P : (2 * k + n_in_bank) * P
                    ].rearrange("p b (s f) -> p s b f", s=n_in_bank),
                    in_=banks[(ps, k)][:, : n_in_bank * 256].rearrange(
                        "p (s b f) -> p s b f", s=n_in_bank, b=2
                    ),
                )
            add(("evac", ps, k), est, deps, em_evac)

            def em_store(ps=ps, k=k, n_in_bank=n_in_bank):
                nc.sync.dma_start(
                    out=outd[
                        ps * 2 * P : (ps + 1) * 2 * P,
                        2 * k * P : (2 * k + n_in_bank) * P,
                    ].rearrange("(blk p) f -> p blk f", p=P),
                    in_=OUTr[
                        :, 2 * ps : 2 * ps + 2, 2 * k * P : (2 * k + n_in_bank) * P
                    ],
                )
            add(("store", ps, k), est + 650.0, [("evac", ps, k)], em_store)

        # topological emission in estimated-start order
        events = {}
        dependents = {}
        for eid, est, deps, emit in raw_events:
            events[eid] = {"est": est, "deps": [], "emit": emit}
        for eid, est, deps, emit in raw_events:
            for d in deps:
                assert d in events, (eid, d)
                events[eid]["deps"].append(d)
                dependents.setdefault(d, []).append(eid)
        ndeps = {eid: len(ev["deps"]) for eid, ev in events.items()}
        heap = []
        ctr = 0
        for eid, ev in events.items():
            if ndeps[eid] == 0:
                heapq.heappush(heap, (ev["est"], ctr, eid))
                ctr += 1
        emitted = 0
        while heap:
            _, _, eid = heapq.heappop(heap)
            events[eid]["emit"]()
            emitted += 1
            for dep in dependents.get(eid, []):
                ndeps[dep] -= 1
                if ndeps[dep] == 0:
                    heapq.heappush(heap, (events[dep]["est"], ctr, dep))
                    ctr += 1
        assert emitted == len(events), (emitted, len(events))

    nc.finalize()  # run the bacc pass pipeline (wait splitting, regalloc, ...)
    return nc


def kernel(x1, x2, cg_tilde, repids_in1, repids_in2, repids_out, out_dim):
    x1 = np.ascontiguousarray(np.asarray(x1, dtype=np.float32))
    x2 = np.ascontiguousarray(np.asarray(x2, dtype=np.float32))
    cg = np.asarray(cg_tilde, dtype=np.float32)
    r1 = np.asarray(repids_in1).astype(np.int64)
    r2 = np.asarray(repids_in2).astype(np.int64)
    ro = np.asarray(repids_out).astype(np.int64)
    out_dim = int(np.asarray(out_dim))

    B, in_dim = x1.shape
    terms = None
    if (
        B % (N_CORES * 2 * P) == 0
        and in_dim % P == 0
        and out_dim % P == 0
        and x2.shape == x1.shape
    ):
        terms = _detect_plan(r1, r2, ro, cg, in_dim, out_dim)
    if terms is None:
        return _numpy_fallback(x1, x2, cg, r1, r2, ro, out_dim)

    b_shard = B // N_CORES
    key = (B, in_dim, out_dim, np.asarray(terms, dtype=np.float64).tobytes())
    nc = _CACHE.get(key)
    if nc is None:
        nc = _build_program(terms, b_shard, in_dim, out_dim)
        _CACHE[key] = nc

    in_maps = [
        {
            "x1": x1[i * b_shard : (i + 1) * b_shard],
            "x2": x2[i * b_shard : (i + 1) * b_shard],
        }
        for i in range(N_CORES)
    ]
    res = run_bass_kernel_spmd(nc, in_maps, core_ids=list(range(N_CORES)))
    return np.concatenate([res.results[i]["out"] for i in range(N_CORES)], axis=0)


# revision 25
# speedup vs baseline: 1.1640x; 1.0878x over previous
"""CG coupler (segment_reduce) Trainium2 kernel.

out[b, ro[t]] += x1[b, r1[t]] * x2[b, r2[t]] * cg[t]   for t in range(T)

The CG index tables produced by the coupler have a rigid structure: T splits
into runs of exactly 128 consecutive indices (the channel dimension) that are
128-aligned in all three tensors, with a constant coefficient per run.  Each
run is therefore one dense slot-level FMA:

    out[:, so*128:(so+1)*128] += c * x1[:, s1*128:...] * x2[:, s2*128:...]

We detect that structure from the runtime index arrays on the host and bake it
into the Bass program.  Per core (batch is data-parallel across 8 cores):

  - inputs stream in per (pass, column-group) so products can start early
  - the distinct (s1,s2) slot products are computed in fp32, split between
    the DVE and Pool engines by a running load-balance
  - per-term scaled-identity matmuls accumulate into PSUM; operands are
    bitcast to float32r, which the PE runs at 1 cycle/row for moving size
    >= 256 (plain fp32 runs at 4 cycles/row)
  - matmuls for one output slot are issued contiguously (start on first,
    stop on last), so no PSUM-zeroing matmuls are needed
  - the Act engine evacuates each PSUM bank to SBUF; the bank's columns are
    then DMA'd straight to DRAM
"""

import sys

for _p in ("/opt/trn_rl_repo",):
    if _p not in sys.path:
        sys.path.insert(0, _p)

from contextlib import ExitStack

import numpy as np

import concourse.bass as bass
import concourse.mybir as mybir
import concourse.tile as tile
from concourse import bacc
from concourse.bass_utils import run_bass_kernel_spmd

N_CORES = 8
P = 128
F32 = mybir.dt.float32
F32R = mybir.dt.float32r
BF16 = mybir.dt.bfloat16

_CACHE: dict = {}


def _detect_plan(r1, r2, ro, cg, in_dim, out_dim):
    """Return list of (s1, s2, so, c) slot terms, or None if the index tables
    don't have the aligned 128-run structure."""
    T = len(cg)
    if T % P != 0 or len(r1) != T or len(r2) != T or len(ro) != T:
        return None
    d1 = np.diff(r1)
    d2 = np.diff(r2)
    do = np.diff(ro)
    brk = np.where(~((d1 == 1) & (d2 == 1) & (do == 1)))[0] + 1
    starts = np.concatenate([[0], brk])
    ends = np.concatenate([brk, [T]])
    if not np.all(ends - starts == P):
        return None
    a0, b0, o0 = r1[starts], r2[starts], ro[starts]
    if (a0 % P).any() or (b0 % P).any() or (o0 % P).any():
        return None
    if a0.max() + P > in_dim or b0.max() + P > in_dim or o0.max() + P > out_dim:
        return None
    cg2 = np.asarray(cg).reshape(-1, P)
    if not np.all(cg2 == cg2[:, :1]):
        return None
    return list(
        zip(
            (a0 // P).tolist(),
            (b0 // P).tolist(),
            (o0 // P).tolist(),
            cg2[:, 0].astype(np.float64).tolist(),
        )
    )


def _numpy_fallback(x1, x2, cg, r1, r2, ro, out_dim):
    out = np.zeros((x1.shape[0], out_dim), dtype=x1.dtype)
    prod = x1[:, r1] * x2[:, r2] * cg[None, :].astype(x1.dtype)
    np.add.at(out, (slice(None), ro), prod)
    return out


# cost-model engine-busy estimates (ns) for [128, N]-free elementwise ops
def _dve_tt(free):  # bf16 tensor_tensor, 2x_1p
    return free * 1.0417 * 0.5 + 60.0


def _pool_tt(free):  # tensor_tensor; Pool gets no DVE 2x modes, 0.42 sw eff
    return free * 0.8333 / 0.42 + 30.0


def _dve_conv(free):  # fp32->bf16 tensor_copy, 2x_2p
    return free * 1.0417 * 0.5 + 60.0


def _act_conv(free):  # fp32->bf16 activation copy
    return free * 0.8333 + 185.0


def _pool_conv(free):  # fp32->bf16 copy on gpsimd (0.6 default sw efficiency)
    return free * 0.8333 / 0.6 + 30.0


_PLAN_CFG = {
    "act_vt0": 2200.0,
    "dve_conv_shadow": 1.0,
    "pool_conv": True,
    "work_conserve": False,  # producers: prefer the idle engine
    "act_conv_ps1": False,  # force pass>=1 conversions onto Act
    "chunks0": [1, 1, 2, 4, 4, 4],  # pass-0 load chunk sizes (slots)
    "chunks1": [4, 4, 2, 2, 2, 2],  # later passes
    "act_conv_ps0_from": 3,  # pass-0 chunks >= this index convert on Act
    "n_combine": 10,  # mirrored pairs folded into S+- combines
}


_ACT_SID_NS = 292.0
_ACT_EVAC_NS = 612.0
_MM_NS = 107.0  # bf16 matmul, 256 moving rows

SLOTS_PER_GROUP = 4  # column-group granularity for input DMA (512 cols)


def _mirror_plan(pairs):
    """Split terms into direct terms and mirror-combined terms.

    Returns (direct, combined, combines) where
      direct:   list of (pair, so, c)             -> rhs = product(pair)
      combined: list of (upair, sign, so, c)      -> rhs = S_sign(upair)
      combines: list of (upair, sign)             -> S_sign = pr_ab + sign*pr_ba
    """
    direct, combined, combines = [], [], set()
    done = set()
    for (a, b), tl in pairs.items():
        if (a, b) in done:
            continue
        if a == b or (b, a) not in pairs:
            done.add((a, b))
            for so, c in tl:
                direct.append(((a, b), so, c))
            continue
        d1 = dict(tl)
        d2 = dict(pairs[(b, a)])
        done.add((a, b))
        done.add((b, a))
        if set(d1) != set(d2):
            for so, c in d1.items():
                direct.append(((a, b), so, c))
            for so, c in d2.items():
                direct.append(((b, a), so, c))
            continue
        ok = all(abs(abs(d1[so]) - abs(d2[so])) <= 1e-5 * abs(d1[so]) for so in d1)
        if not ok:
            for so, c in d1.items():
                direct.append(((a, b), so, c))
            for so, c in d2.items():
                direct.append(((b, a), so, c))
            continue
        up = (a, b) if a < b else (b, a)
        da, db = (d1, d2) if a < b else (d2, d1)
        for so in da:
            sign = 1 if da[so] * db[so] > 0 else -1
            combined.append((up, sign, so, da[so]))
            combines.add((up, sign))
    return direct, combined, sorted(combines)


def _build_program(terms, b_shard, in_dim, out_dim):
    """Build the per-core Bass program. Every core runs the same program on
    its own batch shard (data-parallel, no collectives).

    v7: inputs are converted to bf16 per chunk (staging pool), pair products
    and mirror-combines run in bf16 on DVE+Pool (2x modes), per-term
    scaled-identity bf16 matmuls accumulate in PSUM (1 cycle/row), and
    mirrored pairs are folded (c*pr_ab + (+-c)*pr_ba = c*(pr_ab +- pr_ba))
    to halve the matmul count.  All engine queues are emitted in
    estimated-execution-time order from a host-side list-scheduling plan.
    """
    nblk = b_shard // P
    assert nblk % 2 == 0
    n_passes = nblk // 2
    n_so = out_dim // P
    n_s_in = in_dim // P
    n_banks = (n_so + 1) // 2

    def pass_chunks(ps):
        sizes = (_PLAN_CFG["chunks0"] if ps == 0 else _PLAN_CFG["chunks1"])
        chunks, s = [], 0
        for sz in sizes:
            if s >= n_s_in:
                break
            e = min(s + sz, n_s_in)
            chunks.append(list(range(s, e)))
            s = e
        while s < n_s_in:
            e = min(s + SLOTS_PER_GROUP, n_s_in)
            chunks.append(list(range(s, e)))
            s = e
        return chunks

    # --- host-side plan -----------------------------------------------------
    # load completion estimates (serial DMA engines, ~0.36 B/ns, ~1.4us fill)
    load_done = {}  # (ps, tensor, chunk_idx) -> t ; also (ps, slot) -> t
    t = 1400.0
    for ps in range(n_passes):
        for ci, chunk in enumerate(pass_chunks(ps)):
            dur = 2 * P * len(chunk) * P * 4 / 0.36
            t += dur
            load_done[(ps, 0, ci)] = t
            t += dur
            load_done[(ps, 1, ci)] = t

    pairs: dict = {}
    for s1, s2, so, c in terms:
        pairs.setdefault((s1, s2), []).append((so, c))

    def grp_w(p):
        return max(p[0], p[1])
    # selective mirror-combining: PE is the end-binding engine, so folding a
    # mirrored pair (c*pr_ab + (+-c)*pr_ba -> c*S_sign) saves PE matmuls at
    # the cost of one DVE/Pool tensor_tensor. Only worth it for pairs whose
    # products land early (producer slack); cap via _PLAN_CFG["n_combine"].
    direct_all, combined_all, _ = _mirror_plan(pairs)
    n_comb = _PLAN_CFG["n_combine"]
    upairs = sorted({up for up, sign, so, c in combined_all},
                    key=lambda up: max(grp_w(up), grp_w((up[1], up[0]))))
    chosen = set(upairs[:n_comb])
    direct = list(direct_all)
    combined, combines = [], set()
    for up, sign, so, c in combined_all:
        if up in chosen:
            combined.append((up, sign, so, c))
            combines.add((up, sign))
        else:
            direct.append((up, so, c))
            direct.append(((up[1], up[0]), so,
                           c if sign > 0 else -c))
    combines = sorted(combines)

    # unified dependency-driven list scheduler: convs and products are
    # dispatched in global ready order (interleaved!), each to the engine
    # that finishes it earliest. Scheduling convs phase-first would push one
    # engine's clock far ahead and starve it of product work.
    import heapq as _hq

    vt = {"dve": 0.0, "pool": 300.0, "act": _PLAN_CFG["act_vt0"]}
    done = {}
    assign = {}
    conv_done = {}  # (ps, tensor, slot) -> t
    heap = []
    for ps in range(n_passes):
        for ci, chunk in enumerate(pass_chunks(ps)):
            free = 2 * len(chunk) * P
            for tn in (0, 1):
                _hq.heappush(
                    heap,
                    (load_done[(ps, tn, ci)], 0, ("conv", ps, tn, ci),
                     {"free": free, "chunk": chunk}),
                )
    comb_of_prod = {}
    for up, sign in combines:
        for ps in range(n_passes):
            for pp in (up, (up[1], up[0])):
                comb_of_prod.setdefault(("prod", ps, pp), []).append(
                    ("comb", ps, up, sign)
                )
    comb_deps = {}
    comb_ready = {}
    for ps in range(n_passes):
        for up, sign in combines:
            comb_deps[("comb", ps, up, sign)] = 2
            comb_ready[("comb", ps, up, sign)] = 0.0
    prod_deps = {}
    for ps in range(n_passes):
        for p in pairs:
            prod_deps[("prod", ps, p)] = 2
    chunk_idx = {}
    for ps in range(n_passes):
        for ci, chunk in enumerate(pass_chunks(ps)):
            for s in chunk:
                chunk_idx[(ps, s)] = ci
    waiters = {}
    for ps in range(n_passes):
        for p in pairs:
            waiters.setdefault(("conv", ps, 0, chunk_idx[(ps, p[0])]), []).append(
                ("prod", ps, p)
            )
            waiters.setdefault(("conv", ps, 1, chunk_idx[(ps, p[1])]), []).append(
                ("prod", ps, p)
            )
    prod_ready = {k: 0.0 for k in prod_deps}
    seq = 1
    while heap:
        ready, _, key, meta = _hq.heappop(heap)
        if key[0] == "conv":
            if (
                key[1] == 0
                and key[3] >= _PLAN_CFG["act_conv_ps0_from"]
            ) or (_PLAN_CFG["act_conv_ps1"] and key[1] >= 1):
                cand = [
                    ("act", max(ready, vt["act"]) + _act_conv(meta["free"]),
                     _act_conv(meta["free"]))
                ]
            else:
                cand = [
                    ("act", max(ready, vt["act"]) + _act_conv(meta["free"]),
                     _act_conv(meta["free"])),
                    ("dve",
                     max(ready, vt["dve"])
                     + _dve_conv(meta["free"]) * _PLAN_CFG["dve_conv_shadow"],
                     _dve_conv(meta["free"])),
                ]
                if _PLAN_CFG["pool_conv"]:
                    cand.append(
                        ("pool", max(ready, vt["pool"]) + _pool_conv(meta["free"]),
                         _pool_conv(meta["free"]))
                    )
        else:  # prod or comb: a [128, 256] tensor_tensor on DVE or Pool
            cand = [
                ("dve", max(ready, vt["dve"]) + _dve_tt(2 * P), _dve_tt(2 * P)),
                ("pool", max(ready, vt["pool"]) + _pool_tt(2 * P), _pool_tt(2 * P)),
            ]
            if _PLAN_CFG["work_conserve"]:
                # prefer an engine that would otherwise sit idle
                idle = [c for c in cand if vt[c[0]] <= ready]
                if idle:
                    cand = idle
        eng, fin, cost = min(cand, key=lambda c: c[1])
        fin = max(ready, vt[eng]) + cost
        vt[eng] = fin
        assign[key] = eng
        done[key] = fin
        if key[0] == "conv":
            _, ps, tn, ci = key
            for s in pass_chunks(ps)[ci]:
                conv_done[(ps, tn, s)] = fin
            for w in waiters.get(key, []):
                prod_ready[w] = max(prod_ready[w], fin)
                prod_deps[w] -= 1
                if prod_deps[w] == 0:
                    seq += 1
                    _hq.heappush(heap, (prod_ready[w], seq, w, None))
        elif key[0] == "prod":
            for w in comb_of_prod.get(key, []):
                comb_ready[w] = max(comb_ready[w], fin)
                comb_deps[w] -= 1
                if comb_deps[w] == 0:
                    seq += 1
                    _hq.heappush(heap, (comb_ready[w], seq, w, None))

    # per-pass slot groups: rhs item for each term, slot ordered by the
    # latest rhs completion; PE progress estimate gives evac/store order
    slot_plans = []  # per pass: list of (slot, [(rhs_key, c), ...])
    cvals_first_use = {}
    evac_est = []  # (est, ps, bank)
    pe_vt = 0.0  # PE progress continues across passes
    for ps in range(n_passes):
        rhs_of = {}
        for p, so, c in direct:
            rhs_of.setdefault(so, []).append((("prod", ps, p), c))
        for up, sign, so, c in combined:
            rhs_of.setdefault(so, []).append((("comb", ps, up, sign), c))
        key_of = {
            so: max(done[rk] for rk, _ in tl) for so, tl in rhs_of.items()
        }
        order = sorted(rhs_of, key=lambda so: (key_of[so], so))
        slot_plan = []
        bank_seen = [0] * n_banks
        for so in order:
            tl = sorted(rhs_of[so], key=lambda rc: done[rc[0]])
            slot_plan.append((so, tl))
            for rk, c in tl:
                pe_vt = max(pe_vt, done[rk]) + _MM_NS
                cvals_first_use.setdefault(c, len(cvals_first_use))
            k = so // 2
            bank_seen[k] += 1
            if bank_seen[k] == (2 if 2 * k + 1 < n_so else 1):
                evac_est.append((pe_vt + 100.0, ps, k))
        slot_plans.append(slot_plan)

    # --- emit -------------------------------------------------------------
    # The Tile framework derives dependencies from program order, so the
    # emission stream must be causally ordered (producers before consumers).
    # Emit a single global stream: a heap ordered by estimated start time,
    # popping events only once their dependencies have been emitted.
    import heapq

    nc = bacc.Bacc("TRN2", target_bir_lowering=False, debug=False)
    x1d = nc.dram_tensor("x1", [b_shard, in_dim], F32, kind="ExternalInput").ap()
    x2d = nc.dram_tensor("x2", [b_shard, in_dim], F32, kind="ExternalInput").ap()
    outd = nc.dram_tensor("out", [b_shard, out_dim], F32, kind="ExternalOutput").ap()

    with tile.TileContext(nc) as tc, ExitStack() as ctx:
        const_p = ctx.enter_context(tc.tile_pool(name="const", bufs=1))
        big_p = ctx.enter_context(tc.tile_pool(name="big", bufs=1))
        stage_p = ctx.enter_context(tc.tile_pool(name="stage", bufs=12))
        prod_p = ctx.enter_context(tc.tile_pool(name="prod", bufs=96))
        psum_p = ctx.enter_context(tc.tile_pool(name="psum", bufs=8, space="PSUM"))

        ident = const_p.tile([P, P], F32, tag="ident")
        nc.gpsimd.memset(ident[:], 0.0)
        nc.gpsimd.affine_select(
            out=ident[:],
            in_=ident[:],
            compare_op=mybir.AluOpType.not_equal,
            fill=1.0,
            base=0,
            pattern=[[-1, P]],
            channel_multiplier=1,
        )

        X1B = big_p.tile([P, nblk * in_dim], BF16, tag="X1B")
        X2B = big_p.tile([P, nblk * in_dim], BF16, tag="X2B")
        OUT = big_p.tile([P, nblk * out_dim], F32, tag="OUT")
        XBr = [
            X1B[:].rearrange("p (blk f) -> p blk f", blk=nblk),
            X2B[:].rearrange("p (blk f) -> p blk f", blk=nblk),
        ]
        OUTr = OUT[:].rearrange("p (blk f) -> p blk f", blk=nblk)

        # PSUM bank tiles, pass-major so pass p+1's bank k aliases pass p's
        banks = {}
        for ps in range(n_passes):
            for k in range(n_banks):
                bk = psum_p.tile([P, 512], F32, tag="bank")
                banks[(ps, k)] = bk

        sids = {}
        for c, i in sorted(cvals_first_use.items(), key=lambda kv: kv[1]):
            t_ = const_p.tile([P, P], BF16, tag=f"sid{i}")
            sids[c] = t_

        # ---- event graph ---------------------------------------------------
        raw_events = []  # (eid, est, deps, emit); deps wired after collection

        def add(eid, est, deps, emit):
            raw_events.append((eid, est, deps, emit))

        chunk_of_slot = {}
        for ps in range(n_passes):
            for ci, chunk in enumerate(pass_chunks(ps)):
                for s in chunk:
                    chunk_of_slot[(ps, s)] = ci

        # sids: emit early, ordered by first use (Act)
        for c, i in sorted(cvals_first_use.items(), key=lambda kv: kv[1]):
            def em_sid(c=c):
                nc.scalar.activation(
                    out=sids[c][:],
                    in_=ident[:],
                    func=mybir.ActivationFunctionType.Copy,
                    scale=float(c),
                )
            add(("sid", c), 500.0 + 40.0 * i, [], em_sid)

        # loads (SP queue); explicit WAR dep on the conv 12 loads back
        load_seq = []
        for ps in range(n_passes):
            for ci, chunk in enumerate(pass_chunks(ps)):
                for tn in (0, 1):
                    load_seq.append((ps, ci, tn))
        stages = {}
        for gi, (ps, ci, tn) in enumerate(load_seq):
            chunk = pass_chunks(ps)[ci]
            cols = slice(chunk[0] * P, (chunk[-1] + 1) * P)
            w = (chunk[-1] + 1 - chunk[0]) * P
            rows = slice(ps * 2 * P, (ps + 1) * 2 * P)
            xd = x1d if tn == 0 else x2d
            dur = 2 * P * w * 4 / 0.36
            deps = []
            if gi >= 12:
                deps.append(("conv",) + load_seq[gi - 12])
            def em_load(ps=ps, ci=ci, tn=tn, cols=cols, w=w, rows=rows, xd=xd):
                st = stage_p.tile([P, 2, SLOTS_PER_GROUP * P], F32, tag="stage")
                nc.sync.dma_start(
                    out=st[:, :, :w],
                    in_=xd[rows, cols].rearrange("(blk p) f -> p blk f", p=P),
                )
                stages[(ps, tn, ci)] = st
            add(("load", ps, ci, tn), load_done[(ps, tn, ci)] - dur, deps, em_load)

        # conversions fp32 -> bf16 into the big bf16 tiles
        for ps in range(n_passes):
            for ci, chunk in enumerate(pass_chunks(ps)):
                cols = slice(chunk[0] * P, (chunk[-1] + 1) * P)
                w = (chunk[-1] + 1 - chunk[0]) * P
                for tn in (0, 1):
                    key = ("conv", ps, ci, tn)
                    eng = assign[("conv", ps, tn, ci)]
                    def em_conv(ps=ps, ci=ci, tn=tn, cols=cols, w=w, eng=eng):
                        st = stages[(ps, tn, ci)]
                        out_ap = XBr[tn][:, 2 * ps : 2 * ps + 2, cols]
                        if eng == "act":
                            nc.scalar.copy(out=out_ap, in_=st[:, :, :w])
                        elif eng == "pool":
                            nc.gpsimd.tensor_copy(out=out_ap, in_=st[:, :, :w])
                        else:
                            nc.vector.tensor_copy(out=out_ap, in_=st[:, :, :w])
                    add(key, done[("conv", ps, tn, ci)],
                        [("load", ps, ci, tn)], em_conv)

        # pair products (DVE / Pool per plan)
        tiles = {}
        for ps in range(n_passes):
            for p in pairs:
                key = ("prod", ps, p)
                deps = [
                    ("conv", ps, chunk_of_slot[(ps, p[0])], 0),
                    ("conv", ps, chunk_of_slot[(ps, p[1])], 1),
                ]
                eng_name = assign[key]
                def em_prod(ps=ps, p=p, eng_name=eng_name, key=key):
                    pr = prod_p.tile([P, 2 * P], BF16, tag="prod")
                    eng = nc.vector if eng_name == "dve" else nc.gpsimd
                    eng.tensor_tensor(
                        out=pr[:].rearrange("p (b f) -> p b f", b=2),
                        in0=XBr[0][:, 2 * ps : 2 * ps + 2, p[0] * P : (p[0] + 1) * P],
                        in1=XBr[1][:, 2 * ps : 2 * ps + 2, p[1] * P : (p[1] + 1) * P],
                        op=mybir.AluOpType.mult,
                    )
                    tiles[key] = pr
                add(key, done[key] - _dve_tt(2 * P), deps, em_prod)

        # mirror-combine ops (S_sign = pr_ab +- pr_ba)
        for ps in range(n_passes):
            for up, sign in combines:
                key = ("comb", ps, up, sign)
                eng_name = assign[key]
                def em_comb(ps=ps, up=up, sign=sign, eng_name=eng_name, key=key):
                    pr = prod_p.tile([P, 2 * P], BF16, tag="prod")
                    eng = nc.vector if eng_name == "dve" else nc.gpsimd
                    eng.tensor_tensor(
                        out=pr[:].rearrange("p (b f) -> p b f", b=2),
                        in0=tiles[("prod", ps, up)][:].rearrange(
                            "p (b f) -> p b f", b=2
                        ),
                        in1=tiles[("prod", ps, (up[1], up[0]))][:].rearrange(
                            "p (b f) -> p b f", b=2
                        ),
                        op=mybir.AluOpType.add
                        if sign > 0
                        else mybir.AluOpType.subtract,
                    )
                    tiles[key] = pr
                add(key, done[key] - _dve_tt(2 * P),
                    [("prod", ps, up), ("prod", ps, (up[1], up[0]))], em_comb)

        # per-slot matmul groups, evacs, stores
        for ps in range(n_passes):
            for so, tl in slot_plans[ps]:
                k, so_l = divmod(so, 2)
                deps = [rk for rk, _ in tl]
                deps += [("sid", c) for _, c in tl]
                if ps > 0:
                    deps.append(("evac", ps - 1, k))
                def em_slot(ps=ps, so=so, tl=tl, k=k, so_l=so_l):
                    for i, (rk, c) in enumerate(tl):
                        nc.tensor.matmul(
                            out=banks[(ps, k)][:, so_l * 256 : so_l * 256 + 256],
                            lhsT=sids[c][:],
                            rhs=tiles[rk][:],
                            start=(i == 0),
                            stop=(i == len(tl) - 1),
                        )
                add(("slot", ps, so), max(done[rk] for rk, _ in tl),
                    deps, em_slot)
        for est, ps, k in evac_est:
            n_in_bank = 2 if 2 * k + 1 < n_so else 1
            deps = [("slot", ps, 2 * k)]
            if n_in_bank == 2:
                deps.append(("slot", ps, 2 * k + 1))
            def em_evac(ps=ps, k=k, n_in_bank=n_in_bank):
                nc.scalar.copy(
                    out=OUTr[
                        :, 2 * ps : 2 * ps + 2, 2 * k * P : (2 * k + n_in_bank) * P
                    ].rearrange("p b (s f) -> p s b f", s=n_in_bank),
                    in_=banks[(ps, k)][:, : n_in_bank * 256].rearrange(
                        "p (s b f) -> p s b f", s=n_in_bank, b=2
                    ),
                )
            add(("evac", ps, k), est, deps, em_evac)

            store_est = max(est + 650.0, max(load_done.values()) + 1.0)

            def em_store(ps=ps, k=k, n_in_bank=n_in_bank):
                nc.sync.dma_start(
                    out=outd[
                        ps * 2 * P : (ps + 1) * 2 * P,
                        2 * k * P : (2 * k + n_in_bank) * P,
                    ].rearrange("(blk p) f -> p blk f", p=P),
                    in_=OUTr[
                        :, 2 * ps : 2 * ps + 2, 2 * k * P : (2 * k + n_in_bank) * P
                    ],
                )
            add(("store", ps, k), store_est, [("evac", ps, k)], em_store)

        # topological emission in estimated-start order
        events = {}
        dependents = {}
        for eid, est, deps, emit in raw_events:
            events[eid] = {"est": est, "deps": [], "emit": emit}
        for eid, est, deps, emit in raw_events:
            for d in deps:
                assert d in events, (eid, d)
                events[eid]["deps"].append(d)
                dependents.setdefault(d, []).append(eid)
        ndeps = {eid: len(ev["deps"]) for eid, ev in events.items()}
        heap = []
        ctr = 0
        for eid, ev in events.items():
            if ndeps[eid] == 0:
                heapq.heappush(heap, (ev["est"], ctr, eid))
                ctr += 1
        emitted = 0
        while heap:
            _, _, eid = heapq.heappop(heap)
            events[eid]["emit"]()
            emitted += 1
            for dep in dependents.get(eid, []):
                ndeps[dep] -= 1
                if ndeps[dep] == 0:
                    heapq.heappush(heap, (events[dep]["est"], ctr, dep))
                    ctr += 1
        assert emitted == len(events), (emitted, len(events))

    nc.finalize()  # run the bacc pass pipeline (wait splitting, regalloc, ...)
    return nc


def kernel(x1, x2, cg_tilde, repids_in1, repids_in2, repids_out, out_dim):
    x1 = np.ascontiguousarray(np.asarray(x1, dtype=np.float32))
    x2 = np.ascontiguousarray(np.asarray(x2, dtype=np.float32))
    cg = np.asarray(cg_tilde, dtype=np.float32)
    r1 = np.asarray(repids_in1).astype(np.int64)
    r2 = np.asarray(repids_in2).astype(np.int64)
    ro = np.asarray(repids_out).astype(np.int64)
    out_dim = int(np.asarray(out_dim))

    B, in_dim = x1.shape
    terms = None
    if (
        B % (N_CORES * 2 * P) == 0
        and in_dim % P == 0
        and out_dim % P == 0
        and x2.shape == x1.shape
    ):
        terms = _detect_plan(r1, r2, ro, cg, in_dim, out_dim)
    if terms is None:
        return _numpy_fallback(x1, x2, cg, r1, r2, ro, out_dim)

    b_shard = B // N_CORES
    key = (B, in_dim, out_dim, np.asarray(terms, dtype=np.float64).tobytes())
    nc = _CACHE.get(key)
    if nc is None:
        nc = _build_program(terms, b_shard, in_dim, out_dim)
        _CACHE[key] = nc

    in_maps = [
        {
            "x1": x1[i * b_shard : (i + 1) * b_shard],
            "x2": x2[i * b_shard : (i + 1) * b_shard],
        }
        for i in range(N_CORES)
    ]
    res = run_bass_kernel_spmd(nc, in_maps, core_ids=list(range(N_CORES)))
    return np.concatenate([res.results[i]["out"] for i in range(N_CORES)], axis=0)


# revision 29
# speedup vs baseline: 1.1697x; 1.0049x over previous
"""CG coupler (segment_reduce) Trainium2 kernel.

out[b, ro[t]] += x1[b, r1[t]] * x2[b, r2[t]] * cg[t]   for t in range(T)

The CG index tables produced by the coupler have a rigid structure: T splits
into runs of exactly 128 consecutive indices (the channel dimension) that are
128-aligned in all three tensors, with a constant coefficient per run.  Each
run is therefore one dense slot-level FMA:

    out[:, so*128:(so+1)*128] += c * x1[:, s1*128:...] * x2[:, s2*128:...]

We detect that structure from the runtime index arrays on the host and bake it
into the Bass program.  Per core (batch is data-parallel across 8 cores):

  - inputs stream in per (pass, column-group) so products can start early
  - the distinct (s1,s2) slot products are computed in fp32, split between
    the DVE and Pool engines by a running load-balance
  - per-term scaled-identity matmuls accumulate into PSUM; operands are
    bitcast to float32r, which the PE runs at 1 cycle/row for moving size
    >= 256 (plain fp32 runs at 4 cycles/row)
  - matmuls for one output slot are issued contiguously (start on first,
    stop on last), so no PSUM-zeroing matmuls are needed
  - the Act engine evacuates each PSUM bank to SBUF; the bank's columns are
    then DMA'd straight to DRAM
"""

import sys

for _p in ("/opt/trn_rl_repo",):
    if _p not in sys.path:
        sys.path.insert(0, _p)

from contextlib import ExitStack

import numpy as np

import concourse.bass as bass
import concourse.mybir as mybir
import concourse.tile as tile
from concourse import bacc
from concourse.bass_utils import run_bass_kernel_spmd

N_CORES = 8
P = 128
F32 = mybir.dt.float32
F32R = mybir.dt.float32r
BF16 = mybir.dt.bfloat16

_CACHE: dict = {}


def _detect_plan(r1, r2, ro, cg, in_dim, out_dim):
    """Return list of (s1, s2, so, c) slot terms, or None if the index tables
    don't have the aligned 128-run structure."""
    T = len(cg)
    if T % P != 0 or len(r1) != T or len(r2) != T or len(ro) != T:
        return None
    d1 = np.diff(r1)
    d2 = np.diff(r2)
    do = np.diff(ro)
    brk = np.where(~((d1 == 1) & (d2 == 1) & (do == 1)))[0] + 1
    starts = np.concatenate([[0], brk])
    ends = np.concatenate([brk, [T]])
    if not np.all(ends - starts == P):
        return None
    a0, b0, o0 = r1[starts], r2[starts], ro[starts]
    if (a0 % P).any() or (b0 % P).any() or (o0 % P).any():
        return None
    if a0.max() + P > in_dim or b0.max() + P > in_dim or o0.max() + P > out_dim:
        return None
    cg2 = np.asarray(cg).reshape(-1, P)
    if not np.all(cg2 == cg2[:, :1]):
        return None
    return list(
        zip(
            (a0 // P).tolist(),
            (b0 // P).tolist(),
            (o0 // P).tolist(),
            cg2[:, 0].astype(np.float64).tolist(),
        )
    )


def _numpy_fallback(x1, x2, cg, r1, r2, ro, out_dim):
    out = np.zeros((x1.shape[0], out_dim), dtype=x1.dtype)
    prod = x1[:, r1] * x2[:, r2] * cg[None, :].astype(x1.dtype)
    np.add.at(out, (slice(None), ro), prod)
    return out


# cost-model engine-busy estimates (ns) for [128, N]-free elementwise ops
def _dve_tt(free):  # bf16 tensor_tensor, 2x_1p (+ measured per-op overhead)
    return free * 1.0417 * 0.5 + 80.0


def _pool_tt(free):  # tensor_tensor; Pool gets no DVE 2x modes, 0.42 sw eff
    return free * 0.8333 / 0.42 + 30.0


def _dve_conv(free):  # fp32->bf16 tensor_copy, 2x_2p
    return free * 1.0417 * 0.5 + 60.0


def _act_conv(free):  # fp32->bf16 activation copy
    return free * 0.8333 + 185.0


def _pool_conv(free):  # fp32->bf16 copy on gpsimd (0.6 default sw efficiency)
    return free * 0.8333 / 0.6 + 30.0


_PLAN_CFG = {
    "act_vt0": 2200.0,
    "dve_conv_shadow": 1.0,
    "pool_conv": True,
    "work_conserve": False,  # producers: prefer the idle engine
    "act_conv_ps1": False,  # force pass>=1 conversions onto Act
    "chunks0": [1, 1, 2, 4, 4, 4],  # pass-0 load chunk sizes (slots)
    "chunks1": [4, 4, 2, 2, 2, 2],  # later passes
    "act_conv_ps0_from": 3,  # pass-0 chunks >= this index convert on Act
    "n_combine": 10,  # mirrored pairs folded into S+- combines
    "act_conv_ps1_from": 99,  # pass>=1 chunks >= this index convert on Act
    "comb_offset": 0,  # skip the first N mirrored pairs when choosing combines
    "sid_spread": 40.0,  # est spacing between scaled-identity builds on Act
    "pool_evacs": 0,  # how many of the latest-finishing bank evacs go to Pool
    "n_combine_late": 0,  # also fold the latest-arriving mirrored pairs
}


_ACT_SID_NS = 292.0
_ACT_EVAC_NS = 612.0
_MM_NS = 107.0  # bf16 matmul, 256 moving rows

SLOTS_PER_GROUP = 4  # column-group granularity for input DMA (512 cols)


def _mirror_plan(pairs):
    """Split terms into direct terms and mirror-combined terms.

    Returns (direct, combined, combines) where
      direct:   list of (pair, so, c)             -> rhs = product(pair)
      combined: list of (upair, sign, so, c)      -> rhs = S_sign(upair)
      combines: list of (upair, sign)             -> S_sign = pr_ab + sign*pr_ba
    """
    direct, combined, combines = [], [], set()
    done = set()
    for (a, b), tl in pairs.items():
        if (a, b) in done:
            continue
        if a == b or (b, a) not in pairs:
            done.add((a, b))
            for so, c in tl:
                direct.append(((a, b), so, c))
            continue
        d1 = dict(tl)
        d2 = dict(pairs[(b, a)])
        done.add((a, b))
        done.add((b, a))
        if set(d1) != set(d2):
            for so, c in d1.items():
                direct.append(((a, b), so, c))
            for so, c in d2.items():
                direct.append(((b, a), so, c))
            continue
        ok = all(abs(abs(d1[so]) - abs(d2[so])) <= 1e-5 * abs(d1[so]) for so in d1)
        if not ok:
            for so, c in d1.items():
                direct.append(((a, b), so, c))
            for so, c in d2.items():
                direct.append(((b, a), so, c))
            continue
        up = (a, b) if a < b else (b, a)
        da, db = (d1, d2) if a < b else (d2, d1)
        for so in da:
            sign = 1 if da[so] * db[so] > 0 else -1
            combined.append((up, sign, so, da[so]))
            combines.add((up, sign))
    return direct, combined, sorted(combines)


def _build_program(terms, b_shard, in_dim, out_dim):
    """Build the per-core Bass program. Every core runs the same program on
    its own batch shard (data-parallel, no collectives).

    v7: inputs are converted to bf16 per chunk (staging pool), pair products
    and mirror-combines run in bf16 on DVE+Pool (2x modes), per-term
    scaled-identity bf16 matmuls accumulate in PSUM (1 cycle/row), and
    mirrored pairs are folded (c*pr_ab + (+-c)*pr_ba = c*(pr_ab +- pr_ba))
    to halve the matmul count.  All engine queues are emitted in
    estimated-execution-time order from a host-side list-scheduling plan.
    """
    nblk = b_shard // P
    assert nblk % 2 == 0
    n_passes = nblk // 2
    n_so = out_dim // P
    n_s_in = in_dim // P
    n_banks = (n_so + 1) // 2

    def pass_chunks(ps):
        sizes = (_PLAN_CFG["chunks0"] if ps == 0 else _PLAN_CFG["chunks1"])
        chunks, s = [], 0
        for sz in sizes:
            if s >= n_s_in:
                break
            e = min(s + sz, n_s_in)
            chunks.append(list(range(s, e)))
            s = e
        while s < n_s_in:
            e = min(s + SLOTS_PER_GROUP, n_s_in)
            chunks.append(list(range(s, e)))
            s = e
        return chunks

    # --- host-side plan -----------------------------------------------------
    # load completion estimates (serial DMA engines, ~0.36 B/ns, ~1.4us fill)
    load_done = {}  # (ps, tensor, chunk_idx) -> t ; also (ps, slot) -> t
    t = 1400.0
    for ps in range(n_passes):
        for ci, chunk in enumerate(pass_chunks(ps)):
            dur = 2 * P * len(chunk) * P * 4 / 0.36
            t += dur
            load_done[(ps, 0, ci)] = t
            t += dur
            load_done[(ps, 1, ci)] = t

    pairs: dict = {}
    for s1, s2, so, c in terms:
        pairs.setdefault((s1, s2), []).append((so, c))

    def grp_w(p):
        return max(p[0], p[1])
    # selective mirror-combining: PE is the end-binding engine, so folding a
    # mirrored pair (c*pr_ab + (+-c)*pr_ba -> c*S_sign) saves PE matmuls at
    # the cost of one DVE/Pool tensor_tensor. Only worth it for pairs whose
    # products land early (producer slack); cap via _PLAN_CFG["n_combine"].
    direct_all, combined_all, _ = _mirror_plan(pairs)
    n_comb = _PLAN_CFG["n_combine"]
    upairs = sorted({up for up, sign, so, c in combined_all},
                    key=lambda up: max(grp_w(up), grp_w((up[1], up[0]))))
    off = _PLAN_CFG["comb_offset"]
    chosen = set(upairs[off : off + n_comb])
    n_late = _PLAN_CFG["n_combine_late"]
    if n_late:
        chosen |= set(upairs[-n_late:]) - set(upairs[off : off + n_comb])
    direct = list(direct_all)
    combined, combines = [], set()
    for up, sign, so, c in combined_all:
        if up in chosen:
            combined.append((up, sign, so, c))
            combines.add((up, sign))
        else:
            direct.append((up, so, c))
            direct.append(((up[1], up[0]), so,
                           c if sign > 0 else -c))
    combines = sorted(combines)

    # unified dependency-driven list scheduler: convs and products are
    # dispatched in global ready order (interleaved!), each to the engine
    # that finishes it earliest. Scheduling convs phase-first would push one
    # engine's clock far ahead and starve it of product work.
    import heapq as _hq

    vt = {"dve": 0.0, "pool": 300.0, "act": _PLAN_CFG["act_vt0"]}
    done = {}
    assign = {}
    conv_done = {}  # (ps, tensor, slot) -> t
    heap = []
    for ps in range(n_passes):
        for ci, chunk in enumerate(pass_chunks(ps)):
            free = 2 * len(chunk) * P
            for tn in (0, 1):
                _hq.heappush(
                    heap,
                    (load_done[(ps, tn, ci)], 0, ("conv", ps, tn, ci),
                     {"free": free, "chunk": chunk}),
                )
    comb_of_prod = {}
    for up, sign in combines:
        for ps in range(n_passes):
            for pp in (up, (up[1], up[0])):
                comb_of_prod.setdefault(("prod", ps, pp), []).append(
                    ("comb", ps, up, sign)
                )
    comb_deps = {}
    comb_ready = {}
    for ps in range(n_passes):
        for up, sign in combines:
            comb_deps[("comb", ps, up, sign)] = 2
            comb_ready[("comb", ps, up, sign)] = 0.0
    prod_deps = {}
    for ps in range(n_passes):
        for p in pairs:
            prod_deps[("prod", ps, p)] = 2
    chunk_idx = {}
    for ps in range(n_passes):
        for ci, chunk in enumerate(pass_chunks(ps)):
            for s in chunk:
                chunk_idx[(ps, s)] = ci
    waiters = {}
    for ps in range(n_passes):
        for p in pairs:
            waiters.setdefault(("conv", ps, 0, chunk_idx[(ps, p[0])]), []).append(
                ("prod", ps, p)
            )
            waiters.setdefault(("conv", ps, 1, chunk_idx[(ps, p[1])]), []).append(
                ("prod", ps, p)
            )
    prod_ready = {k: 0.0 for k in prod_deps}
    seq = 1
    while heap:
        ready, _, key, meta = _hq.heappop(heap)
        if key[0] == "conv":
            if (
                key[1] == 0
                and key[3] >= _PLAN_CFG["act_conv_ps0_from"]
            ) or (key[1] >= 1 and key[3] >= _PLAN_CFG["act_conv_ps1_from"]):
                cand = [
                    ("act", max(ready, vt["act"]) + _act_conv(meta["free"]),
                     _act_conv(meta["free"]))
                ]
            else:
                cand = [
                    ("act", max(ready, vt["act"]) + _act_conv(meta["free"]),
                     _act_conv(meta["free"])),
                    ("dve",
                     max(ready, vt["dve"])
                     + _dve_conv(meta["free"]) * _PLAN_CFG["dve_conv_shadow"],
                     _dve_conv(meta["free"])),
                ]
                if _PLAN_CFG["pool_conv"]:
                    cand.append(
                        ("pool", max(ready, vt["pool"]) + _pool_conv(meta["free"]),
                         _pool_conv(meta["free"]))
                    )
        else:  # prod or comb: a [128, 256] tensor_tensor on DVE or Pool
            cand = [
                ("dve", max(ready, vt["dve"]) + _dve_tt(2 * P), _dve_tt(2 * P)),
                ("pool", max(ready, vt["pool"]) + _pool_tt(2 * P), _pool_tt(2 * P)),
            ]
            if _PLAN_CFG["work_conserve"]:
                # prefer an engine that would otherwise sit idle
                idle = [c for c in cand if vt[c[0]] <= ready]
                if idle:
                    cand = idle
        eng, fin, cost = min(cand, key=lambda c: c[1])
        fin = max(ready, vt[eng]) + cost
        vt[eng] = fin
        assign[key] = eng
        done[key] = fin
        if key[0] == "conv":
            _, ps, tn, ci = key
            for s in pass_chunks(ps)[ci]:
                conv_done[(ps, tn, s)] = fin
            for w in waiters.get(key, []):
                prod_ready[w] = max(prod_ready[w], fin)
                prod_deps[w] -= 1
                if prod_deps[w] == 0:
                    seq += 1
                    _hq.heappush(heap, (prod_ready[w], seq, w, None))
        elif key[0] == "prod":
            for w in comb_of_prod.get(key, []):
                comb_ready[w] = max(comb_ready[w], fin)
                comb_deps[w] -= 1
                if comb_deps[w] == 0:
                    seq += 1
                    _hq.heappush(heap, (comb_ready[w], seq, w, None))

    # per-pass slot groups: rhs item for each term, slot ordered by the
    # latest rhs completion; PE progress estimate gives evac/store order
    slot_plans = []  # per pass: list of (slot, [(rhs_key, c), ...])
    cvals_first_use = {}
    evac_est = []  # (est, ps, bank)
    pe_vt = 0.0  # PE progress continues across passes
    for ps in range(n_passes):
        rhs_of = {}
        for p, so, c in direct:
            rhs_of.setdefault(so, []).append((("prod", ps, p), c))
        for up, sign, so, c in combined:
            rhs_of.setdefault(so, []).append((("comb", ps, up, sign), c))
        key_of = {
            so: max(done[rk] for rk, _ in tl) for so, tl in rhs_of.items()
        }
        order = sorted(rhs_of, key=lambda so: (key_of[so], so))
        slot_plan = []
        bank_seen = [0] * n_banks
        for so in order:
            tl = sorted(rhs_of[so], key=lambda rc: done[rc[0]])
            slot_plan.append((so, tl))
            for rk, c in tl:
                pe_vt = max(pe_vt, done[rk]) + _MM_NS
                cvals_first_use.setdefault(c, len(cvals_first_use))
            k = so // 2
            bank_seen[k] += 1
            if bank_seen[k] == (2 if 2 * k + 1 < n_so else 1):
                evac_est.append((pe_vt + 100.0, ps, k))
        slot_plans.append(slot_plan)

    # --- emit -------------------------------------------------------------
    # The Tile framework derives dependencies from program order, so the
    # emission stream must be causally ordered (producers before consumers).
    # Emit a single global stream: a heap ordered by estimated start time,
    # popping events only once their dependencies have been emitted.
    import heapq

    nc = bacc.Bacc("TRN2", target_bir_lowering=False, debug=False)
    x1d = nc.dram_tensor("x1", [b_shard, in_dim], F32, kind="ExternalInput").ap()
    x2d = nc.dram_tensor("x2", [b_shard, in_dim], F32, kind="ExternalInput").ap()
    outd = nc.dram_tensor("out", [b_shard, out_dim], F32, kind="ExternalOutput").ap()

    with tile.TileContext(nc) as tc, ExitStack() as ctx:
        const_p = ctx.enter_context(tc.tile_pool(name="const", bufs=1))
        big_p = ctx.enter_context(tc.tile_pool(name="big", bufs=1))
        stage_p = ctx.enter_context(tc.tile_pool(name="stage", bufs=12))
        prod_p = ctx.enter_context(tc.tile_pool(name="prod", bufs=96))
        psum_p = ctx.enter_context(tc.tile_pool(name="psum", bufs=8, space="PSUM"))

        ident = const_p.tile([P, P], F32, tag="ident")
        nc.gpsimd.memset(ident[:], 0.0)
        nc.gpsimd.affine_select(
            out=ident[:],
            in_=ident[:],
            compare_op=mybir.AluOpType.not_equal,
            fill=1.0,
            base=0,
            pattern=[[-1, P]],
            channel_multiplier=1,
        )

        X1B = big_p.tile([P, nblk * in_dim], BF16, tag="X1B")
        X2B = big_p.tile([P, nblk * in_dim], BF16, tag="X2B")
        OUT = big_p.tile([P, nblk * out_dim], F32, tag="OUT")
        XBr = [
            X1B[:].rearrange("p (blk f) -> p blk f", blk=nblk),
            X2B[:].rearrange("p (blk f) -> p blk f", blk=nblk),
        ]
        OUTr = OUT[:].rearrange("p (blk f) -> p blk f", blk=nblk)

        # PSUM bank tiles, pass-major so pass p+1's bank k aliases pass p's
        banks = {}
        for ps in range(n_passes):
            for k in range(n_banks):
                bk = psum_p.tile([P, 512], F32, tag="bank")
                banks[(ps, k)] = bk

        sids = {}
        for c, i in sorted(cvals_first_use.items(), key=lambda kv: kv[1]):
            t_ = const_p.tile([P, P], BF16, tag=f"sid{i}")
            sids[c] = t_

        # ---- event graph ---------------------------------------------------
        raw_events = []  # (eid, est, deps, emit); deps wired after collection

        def add(eid, est, deps, emit):
            raw_events.append((eid, est, deps, emit))

        chunk_of_slot = {}
        for ps in range(n_passes):
            for ci, chunk in enumerate(pass_chunks(ps)):
                for s in chunk:
                    chunk_of_slot[(ps, s)] = ci

        # sids: emit early, ordered by first use (Act)
        for c, i in sorted(cvals_first_use.items(), key=lambda kv: kv[1]):
            def em_sid(c=c):
                nc.scalar.activation(
                    out=sids[c][:],
                    in_=ident[:],
                    func=mybir.ActivationFunctionType.Copy,
                    scale=float(c),
                )
            add(("sid", c), 500.0 + _PLAN_CFG["sid_spread"] * i, [], em_sid)

        # loads (SP queue); explicit WAR dep on the conv 12 loads back
        load_seq = []
        for ps in range(n_passes):
            for ci, chunk in enumerate(pass_chunks(ps)):
                for tn in (0, 1):
                    load_seq.append((ps, ci, tn))
        stages = {}
        for gi, (ps, ci, tn) in enumerate(load_seq):
            chunk = pass_chunks(ps)[ci]
            cols = slice(chunk[0] * P, (chunk[-1] + 1) * P)
            w = (chunk[-1] + 1 - chunk[0]) * P
            rows = slice(ps * 2 * P, (ps + 1) * 2 * P)
            xd = x1d if tn == 0 else x2d
            dur = 2 * P * w * 4 / 0.36
            deps = []
            if gi >= 12:
                deps.append(("conv",) + load_seq[gi - 12])
            def em_load(ps=ps, ci=ci, tn=tn, cols=cols, w=w, rows=rows, xd=xd):
                st = stage_p.tile([P, 2, SLOTS_PER_GROUP * P], F32, tag="stage")
                nc.sync.dma_start(
                    out=st[:, :, :w],
                    in_=xd[rows, cols].rearrange("(blk p) f -> p blk f", p=P),
                )
                stages[(ps, tn, ci)] = st
            add(("load", ps, ci, tn), load_done[(ps, tn, ci)] - dur, deps, em_load)

        # conversions fp32 -> bf16 into the big bf16 tiles
        for ps in range(n_passes):
            for ci, chunk in enumerate(pass_chunks(ps)):
                cols = slice(chunk[0] * P, (chunk[-1] + 1) * P)
                w = (chunk[-1] + 1 - chunk[0]) * P
                for tn in (0, 1):
                    key = ("conv", ps, ci, tn)
                    eng = assign[("conv", ps, tn, ci)]
                    def em_conv(ps=ps, ci=ci, tn=tn, cols=cols, w=w, eng=eng):
                        st = stages[(ps, tn, ci)]
                        out_ap = XBr[tn][:, 2 * ps : 2 * ps + 2, cols]
                        if eng == "act":
                            nc.scalar.copy(out=out_ap, in_=st[:, :, :w])
                        elif eng == "pool":
                            nc.gpsimd.tensor_copy(out=out_ap, in_=st[:, :, :w])
                        else:
                            nc.vector.tensor_copy(out=out_ap, in_=st[:, :, :w])
                    add(key, done[("conv", ps, tn, ci)],
                        [("load", ps, ci, tn)], em_conv)

        # pair products (DVE / Pool per plan)
        tiles = {}
        for ps in range(n_passes):
            for p in pairs:
                key = ("prod", ps, p)
                deps = [
                    ("conv", ps, chunk_of_slot[(ps, p[0])], 0),
                    ("conv", ps, chunk_of_slot[(ps, p[1])], 1),
                ]
                eng_name = assign[key]
                def em_prod(ps=ps, p=p, eng_name=eng_name, key=key):
                    pr = prod_p.tile([P, 2 * P], BF16, tag="prod")
                    eng = nc.vector if eng_name == "dve" else nc.gpsimd
                    eng.tensor_tensor(
                        out=pr[:].rearrange("p (b f) -> p b f", b=2),
                        in0=XBr[0][:, 2 * ps : 2 * ps + 2, p[0] * P : (p[0] + 1) * P],
                        in1=XBr[1][:, 2 * ps : 2 * ps + 2, p[1] * P : (p[1] + 1) * P],
                        op=mybir.AluOpType.mult,
                    )
                    tiles[key] = pr
                add(key, done[key] - _dve_tt(2 * P), deps, em_prod)

        # mirror-combine ops (S_sign = pr_ab +- pr_ba)
        for ps in range(n_passes):
            for up, sign in combines:
                key = ("comb", ps, up, sign)
                eng_name = assign[key]
                def em_comb(ps=ps, up=up, sign=sign, eng_name=eng_name, key=key):
                    pr = prod_p.tile([P, 2 * P], BF16, tag="prod")
                    eng = nc.vector if eng_name == "dve" else nc.gpsimd
                    eng.tensor_tensor(
                        out=pr[:].rearrange("p (b f) -> p b f", b=2),
                        in0=tiles[("prod", ps, up)][:].rearrange(
                            "p (b f) -> p b f", b=2
                        ),
                        in1=tiles[("prod", ps, (up[1], up[0]))][:].rearrange(
                            "p (b f) -> p b f", b=2
                        ),
                        op=mybir.AluOpType.add
                        if sign > 0
                        else mybir.AluOpType.subtract,
                    )
                    tiles[key] = pr
                add(key, done[key] - _dve_tt(2 * P),
                    [("prod", ps, up), ("prod", ps, (up[1], up[0]))], em_comb)

        # per-slot matmul groups, evacs, stores
        for ps in range(n_passes):
            for so, tl in slot_plans[ps]:
                k, so_l = divmod(so, 2)
                deps = [rk for rk, _ in tl]
                deps += [("sid", c) for _, c in tl]
                if ps > 0:
                    deps.append(("evac", ps - 1, k))
                def em_slot(ps=ps, so=so, tl=tl, k=k, so_l=so_l):
                    for i, (rk, c) in enumerate(tl):
                        nc.tensor.matmul(
                            out=banks[(ps, k)][:, so_l * 256 : so_l * 256 + 256],
                            lhsT=sids[c][:],
                            rhs=tiles[rk][:],
                            start=(i == 0),
                            stop=(i == len(tl) - 1),
                        )
                add(("slot", ps, so), max(done[rk] for rk, _ in tl),
                    deps, em_slot)
        evac_rank = {
            (ps, k): i
            for i, (est, ps, k) in enumerate(sorted(evac_est, reverse=True))
        }
        for est, ps, k in evac_est:
            n_in_bank = 2 if 2 * k + 1 < n_so else 1
            deps = [("slot", ps, 2 * k)]
            if n_in_bank == 2:
                deps.append(("slot", ps, 2 * k + 1))
            on_pool = evac_rank[(ps, k)] < _PLAN_CFG["pool_evacs"]
            def em_evac(ps=ps, k=k, n_in_bank=n_in_bank, on_pool=on_pool):
                out_ap = OUTr[
                    :, 2 * ps : 2 * ps + 2, 2 * k * P : (2 * k + n_in_bank) * P
                ].rearrange("p b (s f) -> p s b f", s=n_in_bank)
                in_ap = banks[(ps, k)][:, : n_in_bank * 256].rearrange(
                    "p (s b f) -> p s b f", s=n_in_bank, b=2
                )
                if on_pool:
                    nc.gpsimd.tensor_copy(out=out_ap, in_=in_ap)
                else:
                    nc.scalar.copy(out=out_ap, in_=in_ap)
            add(("evac", ps, k), est, deps, em_evac)

            store_est = max(est + 650.0, max(load_done.values()) + 1.0)

            def em_store(ps=ps, k=k, n_in_bank=n_in_bank):
                nc.sync.dma_start(
                    out=outd[
                        ps * 2 * P : (ps + 1) * 2 * P,
                        2 * k * P : (2 * k + n_in_bank) * P,
                    ].rearrange("(blk p) f -> p blk f", p=P),
                    in_=OUTr[
                        :, 2 * ps : 2 * ps + 2, 2 * k * P : (2 * k + n_in_bank) * P
                    ],
                )
            add(("store", ps, k), store_est, [("evac", ps, k)], em_store)

        # topological emission in estimated-start order
        events = {}
        dependents = {}
        for eid, est, deps, emit in raw_events:
            events[eid] = {"est": est, "deps": [], "emit": emit}
        for eid, est, deps, emit in raw_events:
            for d in deps:
                assert d in events, (eid, d)
                events[eid]["deps"].append(d)
                dependents.setdefault(d, []).append(eid)
        ndeps = {eid: len(ev["deps"]) for eid, ev in events.items()}
        heap = []
        ctr = 0
        for eid, ev in events.items():
            if ndeps[eid] == 0:
                heapq.heappush(heap, (ev["est"], ctr, eid))
                ctr += 1
        emitted = 0
        while heap:
            _, _, eid = heapq.heappop(heap)
            events[eid]["emit"]()
            emitted += 1
            for dep in dependents.get(eid, []):
                ndeps[dep] -= 1
                if ndeps[dep] == 0:
                    heapq.heappush(heap, (events[dep]["est"], ctr, dep))
                    ctr += 1
        assert emitted == len(events), (emitted, len(events))

    nc.finalize()  # run the bacc pass pipeline (wait splitting, regalloc, ...)
    return nc


def kernel(x1, x2, cg_tilde, repids_in1, repids_in2, repids_out, out_dim):
    x1 = np.ascontiguousarray(np.asarray(x1, dtype=np.float32))
    x2 = np.ascontiguousarray(np.asarray(x2, dtype=np.float32))
    cg = np.asarray(cg_tilde, dtype=np.float32)
    r1 = np.asarray(repids_in1).astype(np.int64)
    r2 = np.asarray(repids_in2).astype(np.int64)
    ro = np.asarray(repids_out).astype(np.int64)
    out_dim = int(np.asarray(out_dim))

    B, in_dim = x1.shape
    terms = None
    if (
        B % (N_CORES * 2 * P) == 0
        and in_dim % P == 0
        and out_dim % P == 0
        and x2.shape == x1.shape
    ):
        terms = _detect_plan(r1, r2, ro, cg, in_dim, out_dim)
    if terms is None:
        return _numpy_fallback(x1, x2, cg, r1, r2, ro, out_dim)

    b_shard = B // N_CORES
    key = (B, in_dim, out_dim, np.asarray(terms, dtype=np.float64).tobytes())
    nc = _CACHE.get(key)
    if nc is None:
        nc = _build_program(terms, b_shard, in_dim, out_dim)
        _CACHE[key] = nc

    in_maps = [
        {
            "x1": x1[i * b_shard : (i + 1) * b_shard],
            "x2": x2[i * b_shard : (i + 1) * b_shard],
        }
        for i in range(N_CORES)
    ]
    res = run_bass_kernel_spmd(nc, in_maps, core_ids=list(range(N_CORES)))
    return np.concatenate([res.results[i]["out"] for i in range(N_CORES)], axis=0)
